# revision 15
# baseline (speedup 1.0000x reference)
"""Trainium2 Bass kernel for fused QKV + RoPE + KV-cache + causal GQA attention + o_proj.

Sharding: tensor-parallel over 8 NeuronCores by attention heads.
Core i owns Q heads [4i, 4i+4), KV head i. x is replicated; each core
computes a partial y over its 512 columns of the o_proj contraction;
the host sums the 8 partials (the "all-reduce after o_proj").

All matmuls run in float32r (tf32) on the PE array: ~4x the fp32 rate
with ~3e-4 end-to-end relative error for this problem's value scales.

Per-core structure (phases overlap via Tile dataflow scheduling):
  A: xqkvT[f,t] = wqkvT.T @ xT, psum-accum over 32 c-tiles, 6 psum
     banks (4xQ|K|V). Evictions alternate DVE/ACT (Identity+bias) and
     are emitted before the RoPE arithmetic so banks free inside the
     HAM window. RoPE rotate-half via SBUF->SBUF DMA partition swap.
     K^T and V (PE-transposed to [t,d]) stay SBUF-resident; Q^T spills
     to DRAM scratch.
  B: per (b, h, l_chunk): S^T[m,l] = K^T.T @ Q^T, exp eviction on ACT
     (scale folded), causal staircase by 0/1 mask multiply, denom r[l]
     via all-ones matmul (result pre-broadcast across partitions), PV
     out^T[d,l] = V.T @ P^T, normalize on eviction. Chunks are
     software-pipelined: S/exp/mask of chunk k is emitted before
     R/PV/recip/evict of chunk k-1 so DVE latency never gates the PE.
  C: y[t,e] = attnT.T @ owT, accum over the 4 local head blocks; owT
     is prefetched at the start of B.
"""

import numpy as np

B, L, H, KVH, D, DIM = 4, 1024, 32, 8, 128, 4096
T = B * L
NC = 8
HLOC = H // NC            # 4 q heads per core
FLOC = (HLOC + 2) * D     # 768 local qkv features
CLOC = HLOC * D           # 512 local o_proj contraction cols

_BUILT = {}


def _build():
    """Build + compile the per-core Bass module (cached per process)."""
    if "nc" in _BUILT:
        return _BUILT["nc"]

    from contextlib import ExitStack
    import concourse.bacc as bacc
    import concourse.mybir as mybir
    import concourse.tile as tile
    from concourse.masks import make_identity

    F32 = mybir.dt.float32
    F32R = mybir.dt.float32r
    EXP = mybir.ActivationFunctionType.Exp
    IDENT = mybir.ActivationFunctionType.Identity

    nc = bacc.Bacc("TRN2", target_bir_lowering=False)

    xT = nc.dram_tensor("xT", [DIM, T], F32R, kind="ExternalInput")
    wqkvT = nc.dram_tensor("wqkvT", [DIM, FLOC], F32R, kind="ExternalInput")
    qkvb = nc.dram_tensor("qkvb", [128, 6], F32, kind="ExternalInput")
    csT = nc.dram_tensor("csT", [128, T], F32, kind="ExternalInput")
    ssT = nc.dram_tensor("ssT", [128, T], F32, kind="ExternalInput")
    owT = nc.dram_tensor("owT", [CLOC, DIM], F32R, kind="ExternalInput")
    cmask = nc.dram_tensor("cmask", [128, 896], F32R, kind="ExternalInput")
    y = nc.dram_tensor("y", [T, DIM], F32, kind="ExternalOutput")

    qT_s = nc.dram_tensor("qT_s", [HLOC, 128, T], F32R)  # Q^T spill
    kT_s = nc.dram_tensor("kT_s", [128, T], F32R)
    v_s = nc.dram_tensor("v_s", [128, T // 128, 128], F32R)

    xT_r = xT[:].rearrange("(co p) t -> p co t", p=128)
    wqkvT_r = wqkvT[:].rearrange("(co p) f -> p co f", p=128)
    owT_r = owT[:].rearrange("(ci p) e -> p ci e", p=128)

    NTB = 8          # token blocks of 512 in phase A
    TB = 512
    NCO = DIM // 128  # 32 contraction tiles

    with tile.TileContext(nc) as tc, ExitStack() as top:
        # B's operand pools live below phase A's pools so attention loads
        # can start as soon as the b=0 spills land, mid-phase-A.
        pb_kv = top.enter_context(tc.tile_pool(name="pb_kv", bufs=2))
        pb_q = top.enter_context(tc.tile_pool(name="pb_q", bufs=2))
        pb_c = top.enter_context(tc.tile_pool(name="pb_c", bufs=1))
        kvs = {}
        qts = {}

        def load_kv(b):
            if b in kvs or b >= B:
                return
            kt_b = pb_kv.tile([128, L], F32R, name="kt_b")
            nc.sync.dma_start(out=kt_b, in_=kT_s[:, b * L:(b + 1) * L])
            v_b = pb_kv.tile([128, L // 128, 128], F32R, name="v_b")
            nc.sync.dma_start(out=v_b, in_=v_s[:, b * 8:(b + 1) * 8, :])
            kvs[b] = (kt_b, v_b)

        def load_q(b, h):
            if (b, h) in qts or b >= B:
                return
            qt = pb_q.tile([128, L], F32R, name="qt")
            nc.sync.dma_start(out=qt, in_=qT_s[h, :, b * L:(b + 1) * L])
            qts[(b, h)] = qt

        # ---------------- Phase A: QKV projection + RoPE ----------------
        with (
            tc.tile_pool(name="pa_w", bufs=1) as pa_w,
            tc.tile_pool(name="pa_x", bufs=8) as pa_x,
            tc.tile_pool(name="pa_cs", bufs=3) as pa_cs,
            tc.tile_pool(name="pa_st", bufs=2) as pa_st,
            tc.tile_pool(name="pa_ps", bufs=1, space="PSUM") as pa_ps,
            tc.tile_pool(name="pa_pst", bufs=1, space="PSUM") as pa_pst,
        ):
            w_sb = pa_w.tile([128, NCO, FLOC], F32R)
            w_chunks = [(0, 1), (1, 2), (2, 4), (4, 8)] + [(c, c + 4) for c in range(8, NCO, 4)]
            for wi, (c0, c1) in enumerate(w_chunks):
                eng = nc.sync if wi < 4 else nc.gpsimd
                eng.dma_start(out=w_sb[:, c0:c1, :], in_=wqkvT_r[:, c0:c1, :])
            b_sb = pa_w.tile([128, 6], F32)
            nc.sync.dma_start(out=b_sb, in_=qkvb[:])
            ident_f = pa_w.tile([128, 128], F32)
            make_identity(nc, ident_f)
            ident = pa_w.tile([128, 128], F32R)
            nc.vector.tensor_copy(ident[:], ident_f[:])

            for tb in range(NTB):
                ts_ = slice(tb * TB, (tb + 1) * TB)
                cs_t = pa_cs.tile([128, TB], F32, name="cs_t")
                nc.scalar.dma_start(out=cs_t, in_=csT[:, ts_])
                ss_t = pa_cs.tile([128, TB], F32, name="ss_t")
                nc.scalar.dma_start(out=ss_t, in_=ssT[:, ts_])
                xts = []
                for co in range(NCO):
                    xt = pa_x.tile([128, TB], F32R, name="xt")
                    nc.sync.dma_start(out=xt, in_=xT_r[:, co, ts_])
                    xts.append(xt)
                psf = [pa_ps.tile([128, TB], F32, name=f"psf{f}",
                                  bufs=2 if f == 0 else 1) for f in range(6)]
                for co in range(NCO):
                    for f in range(6):
                        nc.tensor.matmul(
                            psf[f][:],
                            w_sb[:, co, f * 128:(f + 1) * 128],
                            xts[co][:],
                            start=(co == 0),
                            stop=(co == NCO - 1),
                        )
                # evictions first (free psum banks fast), DVE/ACT alternating
                stages = []
                for f in range(5):
                    st = pa_st.tile([128, TB], F32, name=f"stage{f}")
                    if f % 2 == 0:
                        nc.vector.tensor_scalar_add(st[:], psf[f][:], b_sb[:, f:f + 1])
                    else:
                        nc.scalar.activation(st[:], psf[f][:], IDENT,
                                             bias=b_sb[:, f:f + 1], scale=1.0)
                    stages.append(st)
                vst = pa_st.tile([128, TB], F32R, name="vst")
                nc.scalar.activation(vst[:], psf[5][:], IDENT,
                                     bias=b_sb[:, 5:6], scale=1.0)
                # rope on Q heads and K
                for f in range(5):
                    st = stages[f]
                    rot = pa_st.tile([128, TB], F32, name="rot")
                    nc.scalar.dma_start(out=rot[0:64, :], in_=st[64:128, :])
                    nc.scalar.dma_start(out=rot[64:128, :], in_=st[0:64, :])
                    t1 = pa_st.tile([128, TB], F32, name="t1")
                    nc.vector.tensor_mul(t1[:], st[:], cs_t[:])
                    t2 = pa_st.tile([128, TB], F32, name="t2")
                    nc.gpsimd.tensor_mul(t2[:], rot[:], ss_t[:])
                    if f < HLOC:
                        qk_out = pa_st.tile([128, TB], F32R, name="qk_out")
                        nc.vector.tensor_add(qk_out[:], t1[:], t2[:])
                        nc.gpsimd.dma_start(out=qT_s[f, :, ts_], in_=qk_out[:])
                    else:
                        kt_out = pa_st.tile([128, TB], F32R, name="kt_out")
                        nc.vector.tensor_add(kt_out[:], t1[:], t2[:])
                        nc.gpsimd.dma_start(out=kT_s[:, ts_], in_=kt_out[:])
                # V: transpose [d,t] -> [t,d] tiles into resident V
                for j in range(TB // 128):
                    pst = pa_pst.tile([128, 128], F32R, name="pst")
                    nc.tensor.transpose(pst[:], vst[:, j * 128:(j + 1) * 128], ident[:])
                    vtile = pa_st.tile([128, 128], F32R, name="vtile")
                    nc.vector.tensor_copy(vtile[:], pst[:])
                    nc.gpsimd.dma_start(out=v_s[:, tb * 4 + j, :], in_=vtile[:])
                if tb == 1:
                    # b=0 attention operands are fully spilled now
                    load_kv(0)
                    load_q(0, 0)
                    load_q(0, 1)

        # ---------------- Phases B+C setup ----------------
        # Pool order controls SBUF placement: the stack allocator puts the
        # earliest-entered pool at the bottom, over the just-freed weights
        # region (released at phase A's last matmul), so B's first loads do
        # not wait for the tail of A's rope/spill chain (which holds the
        # staging region near the top).
        ow_pool = top.enter_context(tc.tile_pool(name="ow", bufs=1))
        attnT = [ow_pool.tile([128, T], F32R, name=f"attnT{h}") for h in range(HLOC)]
        ow_qs = {}

        def load_ow_quarter(eq):
            if eq in ow_qs or eq >= 4:
                return
            owq = ow_pool.tile([128, HLOC, 1024], F32R, name="owq", bufs=2)
            nc.scalar.dma_start(out=owq, in_=owT_r[:, :, eq * 1024:(eq + 1) * 1024])
            ow_qs[eq] = owq

        load_ow_quarter(0)

        # ---------------- Phase B: attention ----------------
        with (
            tc.tile_pool(name="pb_pt", bufs=18) as pb_pt,
            tc.tile_pool(name="pb_r", bufs=2) as pb_r,
            tc.tile_pool(name="pb_psS", bufs=3, space="PSUM") as pb_psS,
            tc.tile_pool(name="pb_psR", bufs=2, space="PSUM") as pb_psR,
            tc.tile_pool(name="pb_psO", bufs=2, space="PSUM") as pb_psO,
        ):
            cm_sb = pb_c.tile([128, 896], F32R)
            nc.sync.dma_start(out=cm_sb, in_=cmask[:])
            ones_f = pb_c.tile([128, 128], F32)
            nc.vector.memset(ones_f, 1.0)
            ones = pb_c.tile([128, 128], F32R)
            nc.vector.tensor_copy(ones[:], ones_f[:])

            scale = 1.0 / float(np.sqrt(D))

            def emit_front(b, h, lc):
                """S^T matmuls + exp + mask for one l-chunk; returns PT list."""
                load_kv(b)
                load_q(b, h)
                kt_b, v_b = kvs[b]
                qt = qts[(b, h)]
                ls_ = slice(lc * 512, (lc + 1) * 512)
                nmt = 4 * (lc + 1)
                pts = []
                for mt in range(nmt):
                    psS = pb_psS.tile([128, 512], F32, name="psS")
                    nc.tensor.matmul(
                        psS[:],
                        kt_b[:, mt * 128:(mt + 1) * 128],
                        qt[:, ls_],
                        start=True, stop=True,
                    )
                    pt = pb_pt.tile([128, 512], F32R, name="pt")
                    nc.scalar.activation(pt[:], psS[:], EXP, scale=scale)
                    j = mt - (nmt - 4)
                    if j >= 0:
                        off = (3 - j) * 128
                        nc.vector.tensor_mul(pt[:], pt[:], cm_sb[:, off:off + 512])
                    pts.append(pt)
                return pts

            def emit_back(b, h, lc, pts):
                """R + PV matmuls, reciprocal, normalized eviction."""
                nmt = len(pts)
                psR = pb_psR.tile([128, 512], F32, name="psR")
                psO = pb_psO.tile([128, 512], F32, name="psO")
                for mt in range(nmt):
                    nc.tensor.matmul(psR[:], ones[:], pts[mt][:],
                                     start=(mt == 0), stop=(mt == nmt - 1))
                v_b = kvs[b][1]
                for mt in range(nmt):
                    nc.tensor.matmul(psO[:], v_b[:, mt, :], pts[mt][:],
                                     start=(mt == 0), stop=(mt == nmt - 1))
                recip = pb_r.tile([128, 512], F32, name="recip")
                nc.vector.reciprocal_approx_fast(out=recip[:], in_=psR[:])
                nc.vector.tensor_mul(
                    attnT[h][:, b * L + lc * 512: b * L + (lc + 1) * 512],
                    psO[:], recip[:])

            steps = [(b, h, lc) for b in range(B) for h in range(HLOC)
                     for lc in range(2)]
            prev = None
            for k, step in enumerate(steps):
                pts = emit_front(*step)
                # prefetch next step's operands ahead of its S matmuls
                if k + 1 < len(steps):
                    nb, nh, _ = steps[k + 1]
                    load_q(nb, nh)
                    if nh == HLOC - 1:
                        load_kv(nb + 1)
                if prev is not None:
                    emit_back(*prev[0], prev[1])
                prev = (step, pts)
            emit_back(*prev[0], prev[1])

        # ---------------- Phase C: o_proj ----------------
        with (
            tc.tile_pool(name="pc_st", bufs=5) as pc_st,
            tc.tile_pool(name="pc_ps", bufs=3, space="PSUM") as pc_ps,
        ):
            for eq in range(4):
                load_ow_quarter(eq + 1)
                owq = ow_qs[eq]
                for tt in range(T // 128):
                    psY = pc_ps.tile([128, 1024], F32, name="psY")
                    for c in range(HLOC):
                        for eb in range(2):
                            es = slice(eb * 512, (eb + 1) * 512)
                            nc.tensor.matmul(
                                psY[:, es],
                                attnT[c][:, tt * 128:(tt + 1) * 128],
                                owq[:, c, es],
                                start=(c == 0), stop=(c == HLOC - 1),
                            )
                    yst = pc_st.tile([128, 1024], F32, name="yst")
                    nc.vector.tensor_copy(yst[:], psY[:])
                    yeng = nc.sync if tt % 2 == 0 else nc.gpsimd
                    yeng.dma_start(
                        out=y[tt * 128:(tt + 1) * 128, eq * 1024:(eq + 1) * 1024],
                        in_=yst[:])

    nc.compile()
    _BUILT["nc"] = nc
    return nc


def _host_prep(x, cos, sin, qkv_w, qkv_b, o_w):
    """Build the 8 per-core input maps (numpy only)."""
    xT = np.ascontiguousarray(x.T)                      # [DIM, T]
    cosT = np.ascontiguousarray(cos.T)                  # [64, T]
    sinT = np.ascontiguousarray(sin.T)
    cs = np.concatenate([cosT, cosT], axis=0)           # [128, T]
    ss = np.concatenate([-sinT, sinT], axis=0)          # [128, T]
    xm, xn = np.meshgrid(np.arange(128), np.arange(896), indexing="ij")
    cmask = (xn >= xm + 384).astype(np.float32)         # [128, 896]

    maps = []
    for i in range(NC):
        qrows = qkv_w[CLOC * i: CLOC * (i + 1)]                   # [512, DIM]
        krows = qkv_w[H * D + D * i: H * D + D * (i + 1)]         # [128, DIM]
        vrows = qkv_w[(H + KVH) * D + D * i: (H + KVH) * D + D * (i + 1)]
        w_loc = np.concatenate([qrows, krows, vrows], axis=0)     # [768, DIM]
        wqkvT = np.ascontiguousarray(w_loc.T)                     # [DIM, 768]
        b_loc = np.concatenate([
            qkv_b[CLOC * i: CLOC * (i + 1)],
            qkv_b[H * D + D * i: H * D + D * (i + 1)],
            qkv_b[(H + KVH) * D + D * i: (H + KVH) * D + D * (i + 1)],
        ])                                                        # [768]
        b_sb = np.ascontiguousarray(b_loc.reshape(6, 128).T)      # [128, 6]
        owT = np.ascontiguousarray(o_w[:, CLOC * i: CLOC * (i + 1)].T)  # [512, DIM]
        maps.append({
            "xT": xT, "wqkvT": wqkvT, "qkvb": b_sb,
            "csT": cs, "ssT": ss, "owT": owT, "cmask": cmask,
        })
    return maps


def _fallback(x, cos, sin, qkv_w, qkv_b, o_w, k_cache, v_cache,
              batch_index, seq_index):
    """Pure-numpy reference semantics for non-canonical scatter indices."""
    xqkv = (x[0] @ qkv_w.T + qkv_b).reshape(T, H + 2 * KVH, D)
    xqk, xv = xqkv[:, :H + KVH], xqkv[:, H + KVH:]
    x1, x2 = xqk[..., :D // 2], xqk[..., D // 2:]
    c, s = cos[:, None, :], sin[:, None, :]
    xqk = np.concatenate([x1 * c - x2 * s, x2 * c + x1 * s], axis=-1)
    xqk = xqk.astype(np.float32)
    xq, xk = xqk[:, :H], xqk[:, H:]
    kc = np.array(k_cache, copy=True)
    vc = np.array(v_cache, copy=True)
    kc[batch_index, seq_index] = xk
    vc[batch_index, seq_index] = xv
    q = xq.reshape(B, L, H, D)
    out = np.zeros((B, L, H, D), np.float32)
    scale = 1.0 / np.sqrt(D)
    G = H // KVH
    tri = np.tril(np.ones((L, L), bool))
    for b in range(B):
        for h in range(H):
            S = (q[b, :, h] @ kc[b, :, h // G].T) * scale
            S = np.where(tri, S, -np.inf)
            S -= S.max(axis=-1, keepdims=True)
            e = np.exp(S)
            p = e / e.sum(-1, keepdims=True)
            out[b, :, h] = p.astype(np.float32) @ vc[b, :, h // G]
    return (out.reshape(1, T, H * D) @ o_w.T).astype(np.float32)


def kernel(x, cos, sin, qkv_w, qkv_b, o_w, k_cache, v_cache,
           batch_index, seq_index, cu_seqlens_q, cu_seqlens_k):
    x = np.asarray(x, np.float32)
    cos = np.asarray(cos, np.float32)
    sin = np.asarray(sin, np.float32)
    qkv_w = np.asarray(qkv_w, np.float32)
    qkv_b = np.asarray(qkv_b, np.float32)
    o_w = np.asarray(o_w, np.float32)

    bi = np.asarray(batch_index)
    si = np.asarray(seq_index)
    canonical = (
        np.array_equal(bi, np.repeat(np.arange(B, dtype=bi.dtype), L))
        and np.array_equal(si, np.tile(np.arange(L, dtype=si.dtype), B))
    )
    if not canonical:
        return _fallback(x, cos, sin, qkv_w, qkv_b, o_w,
                         np.asarray(k_cache), np.asarray(v_cache), bi, si)

    from concourse.bass_utils import run_bass_kernel_spmd

    nc = _build()
    in_maps = _host_prep(x[0], cos, sin, qkv_w, qkv_b, o_w)
    res = run_bass_kernel_spmd(nc, in_maps, core_ids=list(range(NC)))
    out = res.results[0]["y"]
    for r in res.results[1:]:
        out = out + r["y"]
    return out.reshape(1, T, H * D).astype(np.float32)


# revision 18
# speedup vs baseline: 1.0085x; 1.0085x over previous
"""Trainium2 Bass kernel for fused QKV + RoPE + KV-cache + causal GQA attention + o_proj.

Sharding: tensor-parallel over 8 NeuronCores by attention heads.
Core i owns Q heads [4i, 4i+4), KV head i. x is replicated; each core
computes a partial y over its 512 columns of the o_proj contraction;
the host sums the 8 partials (the "all-reduce after o_proj").

All matmuls run in float32r (tf32) on the PE array: ~4x the fp32 rate
with ~3e-4 end-to-end relative error for this problem's value scales.

Per-core structure (phases overlap via Tile dataflow scheduling):
  A: xqkvT[f,t] = wqkvT.T @ xT, psum-accum over 32 c-tiles, 6 psum
     banks (4xQ|K|V). Evictions alternate DVE/ACT (Identity+bias) and
     are emitted before the RoPE arithmetic so banks free inside the
     HAM window. RoPE rotate-half via SBUF->SBUF DMA partition swap.
     K^T and V (PE-transposed to [t,d]) stay SBUF-resident; Q^T spills
     to DRAM scratch.
  B: per (b, h, l_chunk): S^T[m,l] = K^T.T @ Q^T, exp eviction on ACT
     (scale folded), causal staircase by 0/1 mask multiply, denom r[l]
     via all-ones matmul (result pre-broadcast across partitions), PV
     out^T[d,l] = V.T @ P^T, normalize on eviction. Chunks are
     software-pipelined: S/exp/mask of chunk k is emitted before
     R/PV/recip/evict of chunk k-1 so DVE latency never gates the PE.
  C: y[t,e] = attnT.T @ owT, accum over the 4 local head blocks; owT
     is prefetched at the start of B.
"""

import numpy as np

B, L, H, KVH, D, DIM = 4, 1024, 32, 8, 128, 4096
T = B * L
NC = 8
HLOC = H // NC            # 4 q heads per core
FLOC = (HLOC + 2) * D     # 768 local qkv features
CLOC = HLOC * D           # 512 local o_proj contraction cols

_BUILT = {}


def _build():
    """Build + compile the per-core Bass module (cached per process)."""
    if "nc" in _BUILT:
        return _BUILT["nc"]

    from contextlib import ExitStack
    import concourse.bacc as bacc
    import concourse.mybir as mybir
    import concourse.tile as tile
    from concourse.masks import make_identity

    F32 = mybir.dt.float32
    F32R = mybir.dt.float32r
    EXP = mybir.ActivationFunctionType.Exp
    IDENT = mybir.ActivationFunctionType.Identity

    nc = bacc.Bacc("TRN2", target_bir_lowering=False)

    xT = nc.dram_tensor("xT", [DIM, T], F32R, kind="ExternalInput")
    wqkvT = nc.dram_tensor("wqkvT", [DIM, FLOC], F32R, kind="ExternalInput")
    qkvb = nc.dram_tensor("qkvb", [128, 6], F32, kind="ExternalInput")
    csT = nc.dram_tensor("csT", [128, T], F32, kind="ExternalInput")
    ssT = nc.dram_tensor("ssT", [128, T], F32, kind="ExternalInput")
    owT = nc.dram_tensor("owT", [CLOC, DIM], F32R, kind="ExternalInput")
    cmask = nc.dram_tensor("cmask", [128, 896], F32R, kind="ExternalInput")
    y = nc.dram_tensor("y", [T, DIM], F32, kind="ExternalOutput")

    qT_s = nc.dram_tensor("qT_s", [HLOC, 128, T], F32R)  # Q^T spill
    kT_s = nc.dram_tensor("kT_s", [128, T], F32R)
    v_s = nc.dram_tensor("v_s", [128, T // 128, 128], F32R)

    xT_r = xT[:].rearrange("(co p) t -> p co t", p=128)
    wqkvT_r = wqkvT[:].rearrange("(co p) f -> p co f", p=128)
    owT_r = owT[:].rearrange("(ci p) e -> p ci e", p=128)

    NTB = 8          # token blocks of 512 in phase A
    TB = 512
    NCO = DIM // 128  # 32 contraction tiles

    with tile.TileContext(nc) as tc, ExitStack() as top:
        # B's operand pools live below phase A's pools so attention loads
        # can start as soon as the b=0 spills land, mid-phase-A.
        pb_kv = top.enter_context(tc.tile_pool(name="pb_kv", bufs=2))
        pb_q = top.enter_context(tc.tile_pool(name="pb_q", bufs=2))
        pb_c = top.enter_context(tc.tile_pool(name="pb_c", bufs=1))
        kvs = {}
        qts = {}

        def load_kv(b):
            if b in kvs or b >= B:
                return
            kt_b = pb_kv.tile([128, L], F32R, name="kt_b")
            nc.sync.dma_start(out=kt_b, in_=kT_s[:, b * L:(b + 1) * L])
            v_b = pb_kv.tile([128, L // 128, 128], F32R, name="v_b")
            nc.sync.dma_start(out=v_b, in_=v_s[:, b * 8:(b + 1) * 8, :])
            kvs[b] = (kt_b, v_b)

        def load_q(b, h):
            if (b, h) in qts or b >= B:
                return
            qt = pb_q.tile([128, L], F32R, name="qt")
            nc.sync.dma_start(out=qt, in_=qT_s[h, :, b * L:(b + 1) * L])
            qts[(b, h)] = qt

        # ---------------- Phase A: QKV projection + RoPE ----------------
        with (
            tc.tile_pool(name="pa_w", bufs=1) as pa_w,
            tc.tile_pool(name="pa_x", bufs=8) as pa_x,
            tc.tile_pool(name="pa_cs", bufs=3) as pa_cs,
            tc.tile_pool(name="pa_st", bufs=2) as pa_st,
            tc.tile_pool(name="pa_ps", bufs=1, space="PSUM") as pa_ps,
            tc.tile_pool(name="pa_pst", bufs=1, space="PSUM") as pa_pst,
        ):
            w_sb = pa_w.tile([128, NCO, FLOC], F32R)
            w_chunks = [(0, 1), (1, 2), (2, 4), (4, 8)] + [(c, c + 4) for c in range(8, NCO, 4)]
            for wi, (c0, c1) in enumerate(w_chunks):
                eng = nc.sync if wi < 4 else nc.gpsimd
                eng.dma_start(out=w_sb[:, c0:c1, :], in_=wqkvT_r[:, c0:c1, :])
            b_sb = pa_w.tile([128, 6], F32)
            nc.sync.dma_start(out=b_sb, in_=qkvb[:])
            ident_f = pa_w.tile([128, 128], F32)
            make_identity(nc, ident_f)
            ident = pa_w.tile([128, 128], F32R)
            nc.vector.tensor_copy(ident[:], ident_f[:])

            for tb in range(NTB):
                ts_ = slice(tb * TB, (tb + 1) * TB)
                cs_t = pa_cs.tile([128, TB], F32, name="cs_t")
                nc.scalar.dma_start(out=cs_t, in_=csT[:, ts_])
                ss_t = pa_cs.tile([128, TB], F32, name="ss_t")
                nc.scalar.dma_start(out=ss_t, in_=ssT[:, ts_])
                xts = []
                for co in range(NCO):
                    xt = pa_x.tile([128, TB], F32R, name="xt")
                    nc.sync.dma_start(out=xt, in_=xT_r[:, co, ts_])
                    xts.append(xt)
                psf = [pa_ps.tile([128, TB], F32, name=f"psf{f}",
                                  bufs=2 if f == 0 else 1) for f in range(6)]
                for co in range(NCO):
                    for f in range(6):
                        nc.tensor.matmul(
                            psf[f][:],
                            w_sb[:, co, f * 128:(f + 1) * 128],
                            xts[co][:],
                            start=(co == 0),
                            stop=(co == NCO - 1),
                        )
                # evictions first (free psum banks fast), DVE/ACT alternating
                stages = []
                for f in range(5):
                    st = pa_st.tile([128, TB], F32, name="stage", bufs=5)
                    if f % 2 == 0:
                        nc.vector.tensor_scalar_add(st[:], psf[f][:], b_sb[:, f:f + 1])
                    else:
                        nc.scalar.activation(st[:], psf[f][:], IDENT,
                                             bias=b_sb[:, f:f + 1], scale=1.0)
                    stages.append(st)
                vst = pa_st.tile([128, TB], F32R, name="vst", bufs=1)
                nc.scalar.activation(vst[:], psf[5][:], IDENT,
                                     bias=b_sb[:, 5:6], scale=1.0)
                # rope on Q heads and K: grouped emission so each engine's
                # FIFO has no cross-engine wait ladder (t2s on gpsimd flow
                # back-to-back; spills ride the scalar queue behind the rots)
                rots, t1s, t2s = [], [], []
                for f in range(5):
                    st = stages[f]
                    rot = pa_st.tile([128, TB], F32, name="rot", bufs=5)
                    nc.scalar.dma_start(out=rot[0:64, :], in_=st[64:128, :])
                    nc.scalar.dma_start(out=rot[64:128, :], in_=st[0:64, :])
                    rots.append(rot)
                for f in range(5):
                    t1 = pa_st.tile([128, TB], F32, name="t1", bufs=5)
                    nc.vector.tensor_mul(t1[:], stages[f][:], cs_t[:])
                    t1s.append(t1)
                for f in range(5):
                    t2 = pa_st.tile([128, TB], F32, name="t2", bufs=5)
                    nc.gpsimd.tensor_mul(t2[:], rots[f][:], ss_t[:])
                    t2s.append(t2)
                outs = []
                for f in range(5):
                    o = pa_st.tile([128, TB], F32R, name="qk_out", bufs=5)
                    nc.vector.tensor_add(o[:], t1s[f][:], t2s[f][:])
                    outs.append(o)
                for f in range(5):
                    dst = qT_s[f, :, ts_] if f < HLOC else kT_s[:, ts_]
                    nc.scalar.dma_start(out=dst, in_=outs[f][:])
                # V: transpose [d,t] -> [t,d] tiles into resident V
                for j in range(TB // 128):
                    pst = pa_pst.tile([128, 128], F32R, name="pst")
                    nc.tensor.transpose(pst[:], vst[:, j * 128:(j + 1) * 128], ident[:])
                    vtile = pa_st.tile([128, 128], F32R, name="vtile")
                    nc.vector.tensor_copy(vtile[:], pst[:])
                    nc.gpsimd.dma_start(out=v_s[:, tb * 4 + j, :], in_=vtile[:])
                if tb == 1:
                    # b=0 attention operands are fully spilled now
                    load_kv(0)
                    load_q(0, 0)
                    load_q(0, 1)

        # ---------------- Phases B+C setup ----------------
        # Pool order controls SBUF placement: the stack allocator puts the
        # earliest-entered pool at the bottom, over the just-freed weights
        # region (released at phase A's last matmul), so B's first loads do
        # not wait for the tail of A's rope/spill chain (which holds the
        # staging region near the top).
        ow_pool = top.enter_context(tc.tile_pool(name="ow", bufs=1))
        attnT = [ow_pool.tile([128, T], F32R, name=f"attnT{h}") for h in range(HLOC)]
        ow_qs = {}

        def load_ow_quarter(eq):
            if eq in ow_qs or eq >= 4:
                return
            owq = ow_pool.tile([128, HLOC, 1024], F32R, name="owq", bufs=2)
            nc.scalar.dma_start(out=owq, in_=owT_r[:, :, eq * 1024:(eq + 1) * 1024])
            ow_qs[eq] = owq

        load_ow_quarter(0)

        # ---------------- Phase B: attention ----------------
        with (
            tc.tile_pool(name="pb_pt", bufs=18) as pb_pt,
            tc.tile_pool(name="pb_r", bufs=2) as pb_r,
            tc.tile_pool(name="pb_psS", bufs=3, space="PSUM") as pb_psS,
            tc.tile_pool(name="pb_psR", bufs=2, space="PSUM") as pb_psR,
            tc.tile_pool(name="pb_psO", bufs=3, space="PSUM") as pb_psO,
        ):
            cm_sb = pb_c.tile([128, 896], F32R)
            nc.sync.dma_start(out=cm_sb, in_=cmask[:])
            ones_f = pb_c.tile([128, 128], F32)
            nc.vector.memset(ones_f, 1.0)
            ones = pb_c.tile([128, 128], F32R)
            nc.vector.tensor_copy(ones[:], ones_f[:])

            scale = 1.0 / float(np.sqrt(D))

            def emit_front(b, h, lc):
                """S^T matmuls + exp + mask for one l-chunk; returns PT list."""
                load_kv(b)
                load_q(b, h)
                kt_b, v_b = kvs[b]
                qt = qts[(b, h)]
                ls_ = slice(lc * 512, (lc + 1) * 512)
                nmt = 4 * (lc + 1)
                pts = []
                for mt in range(nmt):
                    psS = pb_psS.tile([128, 512], F32, name="psS")
                    nc.tensor.matmul(
                        psS[:],
                        kt_b[:, mt * 128:(mt + 1) * 128],
                        qt[:, ls_],
                        start=True, stop=True,
                    )
                    pt = pb_pt.tile([128, 512], F32R, name="pt")
                    nc.scalar.activation(pt[:], psS[:], EXP, scale=scale)
                    j = mt - (nmt - 4)
                    if j >= 0:
                        off = (3 - j) * 128
                        nc.vector.tensor_mul(pt[:], pt[:], cm_sb[:, off:off + 512])
                    pts.append(pt)
                return pts

            def emit_back(b, h, lc, pts):
                """R + PV matmuls, reciprocal, normalized eviction."""
                nmt = len(pts)
                psR = pb_psR.tile([128, 512], F32, name="psR")
                psO = pb_psO.tile([128, 512], F32, name="psO")
                for mt in range(nmt):
                    nc.tensor.matmul(psR[:], ones[:], pts[mt][:],
                                     start=(mt == 0), stop=(mt == nmt - 1))
                v_b = kvs[b][1]
                for mt in range(nmt):
                    nc.tensor.matmul(psO[:], v_b[:, mt, :], pts[mt][:],
                                     start=(mt == 0), stop=(mt == nmt - 1))
                recip = pb_r.tile([128, 512], F32, name="recip")
                nc.vector.reciprocal_approx_fast(out=recip[:], in_=psR[:])
                nc.vector.tensor_mul(
                    attnT[h][:, b * L + lc * 512: b * L + (lc + 1) * 512],
                    psO[:], recip[:])

            steps = [(b, h, lc) for b in range(B) for h in range(HLOC)
                     for lc in range(2)]
            prev = None
            for k, step in enumerate(steps):
                pts = emit_front(*step)
                # prefetch next step's operands ahead of its S matmuls
                if k + 1 < len(steps):
                    nb, nh, _ = steps[k + 1]
                    load_q(nb, nh)
                    if nh == HLOC - 1:
                        load_kv(nb + 1)
                if prev is not None:
                    emit_back(*prev[0], prev[1])
                prev = (step, pts)
            emit_back(*prev[0], prev[1])

        # ---------------- Phase C: o_proj ----------------
        with (
            tc.tile_pool(name="pc_st", bufs=5) as pc_st,
            tc.tile_pool(name="pc_ps", bufs=3, space="PSUM") as pc_ps,
        ):
            for eq in range(4):
                load_ow_quarter(eq + 1)
                owq = ow_qs[eq]
                for tt in range(T // 128):
                    psY = pc_ps.tile([128, 1024], F32, name="psY")
                    for c in range(HLOC):
                        for eb in range(2):
                            es = slice(eb * 512, (eb + 1) * 512)
                            nc.tensor.matmul(
                                psY[:, es],
                                attnT[c][:, tt * 128:(tt + 1) * 128],
                                owq[:, c, es],
                                start=(c == 0), stop=(c == HLOC - 1),
                            )
                    yst = pc_st.tile([128, 1024], F32, name="yst")
                    nc.vector.tensor_copy(yst[:], psY[:])
                    yeng = nc.sync if tt % 2 == 0 else nc.gpsimd
                    yeng.dma_start(
                        out=y[tt * 128:(tt + 1) * 128, eq * 1024:(eq + 1) * 1024],
                        in_=yst[:])

    nc.compile()
    _BUILT["nc"] = nc
    return nc


def _host_prep(x, cos, sin, qkv_w, qkv_b, o_w):
    """Build the 8 per-core input maps (numpy only)."""
    xT = np.ascontiguousarray(x.T)                      # [DIM, T]
    cosT = np.ascontiguousarray(cos.T)                  # [64, T]
    sinT = np.ascontiguousarray(sin.T)
    cs = np.concatenate([cosT, cosT], axis=0)           # [128, T]
    ss = np.concatenate([-sinT, sinT], axis=0)          # [128, T]
    xm, xn = np.meshgrid(np.arange(128), np.arange(896), indexing="ij")
    cmask = (xn >= xm + 384).astype(np.float32)         # [128, 896]

    maps = []
    for i in range(NC):
        qrows = qkv_w[CLOC * i: CLOC * (i + 1)]                   # [512, DIM]
        krows = qkv_w[H * D + D * i: H * D + D * (i + 1)]         # [128, DIM]
        vrows = qkv_w[(H + KVH) * D + D * i: (H + KVH) * D + D * (i + 1)]
        w_loc = np.concatenate([qrows, krows, vrows], axis=0)     # [768, DIM]
        wqkvT = np.ascontiguousarray(w_loc.T)                     # [DIM, 768]
        b_loc = np.concatenate([
            qkv_b[CLOC * i: CLOC * (i + 1)],
            qkv_b[H * D + D * i: H * D + D * (i + 1)],
            qkv_b[(H + KVH) * D + D * i: (H + KVH) * D + D * (i + 1)],
        ])                                                        # [768]
        b_sb = np.ascontiguousarray(b_loc.reshape(6, 128).T)      # [128, 6]
        owT = np.ascontiguousarray(o_w[:, CLOC * i: CLOC * (i + 1)].T)  # [512, DIM]
        maps.append({
            "xT": xT, "wqkvT": wqkvT, "qkvb": b_sb,
            "csT": cs, "ssT": ss, "owT": owT, "cmask": cmask,
        })
    return maps


def _fallback(x, cos, sin, qkv_w, qkv_b, o_w, k_cache, v_cache,
              batch_index, seq_index):
    """Pure-numpy reference semantics for non-canonical scatter indices."""
    xqkv = (x[0] @ qkv_w.T + qkv_b).reshape(T, H + 2 * KVH, D)
    xqk, xv = xqkv[:, :H + KVH], xqkv[:, H + KVH:]
    x1, x2 = xqk[..., :D // 2], xqk[..., D // 2:]
    c, s = cos[:, None, :], sin[:, None, :]
    xqk = np.concatenate([x1 * c - x2 * s, x2 * c + x1 * s], axis=-1)
    xqk = xqk.astype(np.float32)
    xq, xk = xqk[:, :H], xqk[:, H:]
    kc = np.array(k_cache, copy=True)
    vc = np.array(v_cache, copy=True)
    kc[batch_index, seq_index] = xk
    vc[batch_index, seq_index] = xv
    q = xq.reshape(B, L, H, D)
    out = np.zeros((B, L, H, D), np.float32)
    scale = 1.0 / np.sqrt(D)
    G = H // KVH
    tri = np.tril(np.ones((L, L), bool))
    for b in range(B):
        for h in range(H):
            S = (q[b, :, h] @ kc[b, :, h // G].T) * scale
            S = np.where(tri, S, -np.inf)
            S -= S.max(axis=-1, keepdims=True)
            e = np.exp(S)
            p = e / e.sum(-1, keepdims=True)
            out[b, :, h] = p.astype(np.float32) @ vc[b, :, h // G]
    return (out.reshape(1, T, H * D) @ o_w.T).astype(np.float32)


def kernel(x, cos, sin, qkv_w, qkv_b, o_w, k_cache, v_cache,
           batch_index, seq_index, cu_seqlens_q, cu_seqlens_k):
    x = np.asarray(x, np.float32)
    cos = np.asarray(cos, np.float32)
    sin = np.asarray(sin, np.float32)
    qkv_w = np.asarray(qkv_w, np.float32)
    qkv_b = np.asarray(qkv_b, np.float32)
    o_w = np.asarray(o_w, np.float32)

    bi = np.asarray(batch_index)
    si = np.asarray(seq_index)
    canonical = (
        np.array_equal(bi, np.repeat(np.arange(B, dtype=bi.dtype), L))
        and np.array_equal(si, np.tile(np.arange(L, dtype=si.dtype), B))
    )
    if not canonical:
        return _fallback(x, cos, sin, qkv_w, qkv_b, o_w,
                         np.asarray(k_cache), np.asarray(v_cache), bi, si)

    from concourse.bass_utils import run_bass_kernel_spmd

    nc = _build()
    in_maps = _host_prep(x[0], cos, sin, qkv_w, qkv_b, o_w)
    res = run_bass_kernel_spmd(nc, in_maps, core_ids=list(range(NC)))
    out = res.results[0]["y"]
    for r in res.results[1:]:
        out = out + r["y"]
    return out.reshape(1, T, H * D).astype(np.float32)
